# revision 32
# baseline (speedup 1.0000x reference)
"""BiRWKV layer kernel for Trainium2 (8 NeuronCores, Bass/Tile).

Problem: x[4,2048,1024] -> rkv = x @ rkv_w.T -> (r,k,v) fwd + bwd,
WKV scan per direction, gate with sigmoid(r), concat, out @ out_w.T.

Strategy (v2):
  - Shard over (batch b, channel-half h): core = 2*b + h. Each core handles
    one batch's 512 fwd + 512 bwd channels end-to-end.
  - Channels-on-partitions layout [c, t]: projections via PE matmul
    (lhsT = W^T tile [c,d], rhs = x^T [c,t]), WKV recurrence via the DVE's
    native tensor_tensor_scan, out-projection partial via PE (contraction
    over local c), summed across cores on host.
  - Unstabilized scan: A_t = lam*A + e^k v, D_t = lam*D + e^k,
    y = (A_{t-1} + e^u e^k v)/(D_{t-1} + e^u e^k). fp32 never overflows for
    this problem's ranges; matches the stabilized reference to ~1e-7.
  - Backward direction = forward scan on time-reversed rhs (stride trick).
  - sigmoid(r)*y = 0.5*(1+tanh(r/2))*y; the 0.5 is folded into out_w.

  v2 performance changes vs v1:
  - x loaded ONCE and kept resident in SBUF across both directions
    (3-buffer ring; only time-chunk 0 is reloaded for the backward pass).
  - Input DMAs batched (1 dispatch/tensor; per-ck only for startup-critical
    tiles) and issued from the ACT engine (own HWDGE queue); output DMAs
    stay on the sync engine queue -> 2 parallel DMA queues.
  - exp/tanh/PSUM-copies on ACT; the fp32 elementwise chain stays on DVE
    (GpSimd only gets tiny carry copies/memsets: its ISA lacks STT/divide,
    its TT ops run at ~2x the DVE cost, and keeping it busy triggers
    power throttling that slows every other engine ~25%).
  - Out-projection in bf16 (Wo, z, outputs) - halves those transfers;
    output partials written bf16, upconverted on host.
  - Final-chunk out-projection accumulates over ct into 8 PSUM banks so the
    tail after the last z is ~8 matmuls instead of a full 32-matmul flush.
"""
import os
import sys
import numpy as np

sys.path.insert(0, "/opt/trn_rl_repo")

import concourse.bass as bass
import concourse.mybir as mybir
from concourse import bacc
import concourse.tile as tile
from concourse.bass_utils import run_bass_kernel_spmd

B, T, C = 4, 2048, 1024
H = C // 2          # channels per core per direction (512)
NCT = H // 128      # c-tiles per direction (4)
TCH = 512           # time chunk
NTC = T // TCH      # t-chunks (4)
F32 = mybir.dt.float32
BF16 = mybir.dt.bfloat16
F32R = mybir.dt.float32r
AF = mybir.ActivationFunctionType
ALU = mybir.AluOpType

MM_DT = {"f32": F32, "f32r": F32R, "bf16": BF16}[
    os.environ.get("BIRWKV_MM_DT", "bf16")]
WO_DT = BF16
OUT_DT = BF16

_compiled = None


def _build():
    nc = bacc.Bacc("TRN2", target_bir_lowering=False, debug=False, num_devices=8)

    xT = nc.dram_tensor("xT", [NTC, 128, 8, TCH], MM_DT, kind="ExternalInput").ap()
    Wkf = nc.dram_tensor("Wkf", [128, 8, H], MM_DT, kind="ExternalInput").ap()
    Wvf = nc.dram_tensor("Wvf", [128, 8, H], MM_DT, kind="ExternalInput").ap()
    Wrf = nc.dram_tensor("Wrf", [128, 8, H], MM_DT, kind="ExternalInput").ap()
    Wkb = nc.dram_tensor("Wkb", [128, 8, H], MM_DT, kind="ExternalInput").ap()
    Wvb = nc.dram_tensor("Wvb", [128, 8, H], MM_DT, kind="ExternalInput").ap()
    Wrb = nc.dram_tensor("Wrb", [128, 8, H], MM_DT, kind="ExternalInput").ap()
    Wof = nc.dram_tensor("Wof", [128, NCT, C], WO_DT, kind="ExternalInput").ap()
    Wob = nc.dram_tensor("Wob", [128, NCT, C], WO_DT, kind="ExternalInput").ap()
    lamf = nc.dram_tensor("lamf", [128, NCT * TCH], F32, kind="ExternalInput").ap()
    lamb = nc.dram_tensor("lamb", [128, NCT * TCH], F32, kind="ExternalInput").ap()
    euf = nc.dram_tensor("euf", [128, NCT], F32, kind="ExternalInput").ap()
    eub = nc.dram_tensor("eub", [128, NCT], F32, kind="ExternalInput").ap()

    outTf = nc.dram_tensor("outTf", [C, T], OUT_DT, kind="ExternalOutput").ap()
    outTb = nc.dram_tensor("outTb", [C, T], OUT_DT, kind="ExternalOutput").ap()

    with tile.TileContext(nc) as tc:
        with (
            tc.tile_pool(name="xp", bufs=1) as xp_pool,
            tc.tile_pool(name="wk", bufs=2) as wk_pool,
            tc.tile_pool(name="wv", bufs=2) as wv_pool,
            tc.tile_pool(name="wr", bufs=2) as wr_pool,
            tc.tile_pool(name="wo", bufs=2) as wo_pool,
            tc.tile_pool(name="lam", bufs=1) as lam_pool,
            tc.tile_pool(name="ew", bufs=2) as ew_pool,
            tc.tile_pool(name="ab", bufs=1) as ab_pool,
            tc.tile_pool(name="zs", bufs=2) as z_pool,
            tc.tile_pool(name="osb", bufs=6) as osb_pool,
            tc.tile_pool(name="pp", bufs=6, space="PSUM") as pp,
            tc.tile_pool(name="po", bufs=2, space="PSUM") as po,
        ):
            # ---- startup DMAs ------------------------------------------
            # x chunks live in a 3-buffer ring (tags xA/xB/xC; time-chunks
            # 0 and 3 share xA).  Chunk 0 + Wkf are the first-matmul
            # critical path: per-ck dispatches (subtile deps let ck0's
            # matmul start while ck1.. are in flight), interleaved on the
            # sync queue.  Wvf/Wrf per-ck on the ACT queue in parallel.
            X_TAG = {0: "xA", 1: "xB", 2: "xC", 3: "xA"}
            x_cur = {}
            x_cur[0] = xp_pool.tile([128, 8, TCH], MM_DT, tag="xA", name="x0")
            x_cur[1] = xp_pool.tile([128, 8, TCH], MM_DT, tag="xB", name="x1")
            x_cur[2] = xp_pool.tile([128, 8, TCH], MM_DT, tag="xC", name="x2")
            wk_t = wk_pool.tile([128, 8, H], MM_DT, tag="wk")
            wv_t = wv_pool.tile([128, 8, H], MM_DT, tag="wv")
            wr_t = wr_pool.tile([128, 8, H], MM_DT, tag="wr")
            wo_t = wo_pool.tile([128, NCT, C], WO_DT, tag="wo")
            for ck in range(8):
                nc.sync.dma_start(wk_t[:, ck], Wkf[:, ck])
                nc.sync.dma_start(x_cur[0][:, ck], xT[0, :, ck])
            for ck in range(8):
                nc.scalar.dma_start(wv_t[:, ck], Wvf[:, ck])
                nc.scalar.dma_start(wr_t[:, ck], Wrf[:, ck])
            # queue order = arrival order.  qAct: wv, wr (ck-streamed,
            # above), wo (~32us), x2 (~55us), lam1/eu1 (backward pass).
            # qSP: wk/x0 (critical), lam0/eu0 (~13us), x1 (~28us), x3.
            nc.scalar.dma_start(wo_t[:], Wof[:])
            lam_t = {}
            eu_t = {}
            lam_t[0] = lam_pool.tile([128, NCT * TCH], F32, tag="lam0", name="lam0")
            nc.sync.dma_start(lam_t[0][:], lamf[:])
            eu_t[0] = lam_pool.tile([128, NCT], F32, tag="eu0", name="eu0")
            nc.sync.dma_start(eu_t[0][:], euf[:])
            nc.sync.dma_start(x_cur[1][:], xT[1])
            nc.scalar.dma_start(x_cur[2][:], xT[2])
            lam_t[1] = lam_pool.tile([128, NCT * TCH], F32, tag="lam1", name="lam1")
            nc.scalar.dma_start(lam_t[1][:], lamb[:])
            eu_t[1] = lam_pool.tile([128, NCT], F32, tag="eu1", name="eu1")
            nc.scalar.dma_start(eu_t[1][:], eub[:])
            x_cur[3] = xp_pool.tile([128, 8, TCH], MM_DT, tag="xD", name="x3")
            nc.sync.dma_start(x_cur[3][:], xT[3])

            def emit_outproj(prev, e0, e1):
                wo_p, z_tiles, outT_p, t0 = prev
                for et in range(e0, e1):
                    esl = slice(et * 128, (et + 1) * 128)
                    o_ps = po.tile([128, TCH], F32, tag="ops")
                    for ct in range(NCT):
                        nc.tensor.matmul(
                            o_ps[:],
                            wo_p[:, ct, esl],
                            z_tiles[ct][:],
                            start=(ct == 0), stop=(ct == NCT - 1),
                        )
                    o_sb = osb_pool.tile([128, TCH], OUT_DT, tag="osb")
                    nc.scalar.copy(o_sb[:], o_ps[:])
                    nc.sync.dma_start(outT_p[et * 128:(et + 1) * 128, t0:t0 + TCH], o_sb[:])

            def elementwise(d, ti, ct, k_ps, v_ps, r_ps):
                """exp/tanh on ACT; pv+scans+num/den+recip on DVE (GpSimd
                has no scalar_tensor_tensor); y + gating + carries on
                GpSimd. Returns the z tile (bf16)."""
                p = ew_pool.tile([128, TCH], F32, tag="p")
                nc.scalar.activation(p[:], k_ps[:], AF.Exp)
                th = ew_pool.tile([128, TCH], F32, tag="th")
                nc.scalar.activation(th[:], r_ps[:], AF.Tanh, scale=0.5)
                pv = ew_pool.tile([128, TCH], F32, tag="pv")
                nc.vector.tensor_mul(pv[:], p[:], v_ps[:])

                a_buf = ab_pool.tile([128, TCH + 1], F32, tag=f"A{ct}", name=f"A{ct}")
                d_buf = ab_pool.tile([128, TCH + 1], F32, tag=f"D{ct}", name=f"D{ct}")
                if ti == 0:
                    nc.gpsimd.memset(a_buf[:, 0:1], 0.0)
                    nc.gpsimd.memset(d_buf[:, 0:1], 0.0)
                else:
                    nc.gpsimd.tensor_copy(a_buf[:, 0:1], a_buf[:, TCH:TCH + 1])
                    nc.gpsimd.tensor_copy(d_buf[:, 0:1], d_buf[:, TCH:TCH + 1])
                lam_sl = lam_t[d][:, ct * TCH:(ct + 1) * TCH]
                nc.vector.tensor_tensor_scan(
                    a_buf[:, 1:TCH + 1], lam_sl, pv[:],
                    a_buf[:, 0:1], ALU.mult, ALU.add)
                nc.vector.tensor_tensor_scan(
                    d_buf[:, 1:TCH + 1], lam_sl, p[:],
                    d_buf[:, 0:1], ALU.mult, ALU.add)

                eu_sl = eu_t[d][:, ct:ct + 1]
                num = ew_pool.tile([128, TCH], F32, tag="num")
                nc.vector.scalar_tensor_tensor(
                    num[:], pv[:], eu_sl, a_buf[:, 0:TCH], ALU.mult, ALU.add)
                den = ew_pool.tile([128, TCH], F32, tag="den")
                nc.vector.scalar_tensor_tensor(
                    den[:], p[:], eu_sl, d_buf[:, 0:TCH], ALU.mult, ALU.add)
                rec = ew_pool.tile([128, TCH], F32, tag="rec")
                nc.vector.reciprocal_approx_fast(rec[:], den[:])
                y = ew_pool.tile([128, TCH], F32, tag="y")
                nc.vector.tensor_mul(y[:], num[:], rec[:])
                z = z_pool.tile([128, TCH], WO_DT, tag=f"z{ct}", name=f"z{ct}")
                nc.vector.scalar_tensor_tensor(
                    z[:], th[:], 1.0, y[:], ALU.add, ALU.mult)
                return z

            def proj_matmuls(ct, x_t, rev, wts):
                dsl = slice(ct * 128, (ct + 1) * 128)
                ps = []
                for w_t in wts:
                    dst = pp.tile([128, TCH], F32, tag="proj", name="ps")
                    for ck in range(8):
                        rhs = x_t[:, ck]
                        if rev:
                            rhs = rhs[:, ::-1]
                        nc.tensor.matmul(
                            dst[:], w_t[:, ck, dsl], rhs,
                            start=(ck == 0), stop=(ck == 7),
                        )
                    ps.append(dst)
                return ps

            prev_out = None
            for d, (Wk, Wv, Wr, Wo, outT_d) in enumerate((
                (Wkf, Wvf, Wrf, Wof, outTf),
                (Wkb, Wvb, Wrb, Wob, outTb),
            )):
                rev = (d == 1)
                if d == 1:
                    # all weights were prefetched into their second ring
                    # slots mid-forward-pass.
                    wk_t = next_w["wk"]
                    wv_t = next_w["wv"]
                    wr_t = next_w["wr"]
                    wo_t = next_w["wo"]

                for ti in range(NTC):
                    t0 = ti * TCH
                    tis = NTC - 1 - ti if rev else ti
                    if d == 0 and ti == 2:
                        # prefetch all backward-pass weights into their free
                        # second ring slots (no dependency wait) so the
                        # pass transition starts with weights in SBUF.
                        next_w = {
                            "wk": wk_pool.tile([128, 8, H], MM_DT, tag="wk",
                                               name="wk2"),
                            "wv": wv_pool.tile([128, 8, H], MM_DT, tag="wv",
                                               name="wv2"),
                            "wr": wr_pool.tile([128, 8, H], MM_DT, tag="wr",
                                               name="wr2"),
                            "wo": wo_pool.tile([128, NCT, C], WO_DT, tag="wo",
                                               name="wo2"),
                        }
                        nc.scalar.dma_start(next_w["wk"][:], Wkb[:])
                        nc.scalar.dma_start(next_w["wv"][:], Wvb[:])
                        nc.scalar.dma_start(next_w["wr"][:], Wrb[:])
                        nc.scalar.dma_start(next_w["wo"][:], Wob[:])
                    x_t = x_cur[tis]
                    final = (d == 1 and ti == NTC - 1)

                    if not final:
                        z_tiles = []
                        # Prev-chunk emits need ALL of the previous z
                        # tiles, and prev-z3 lands barely before this chunk
                        # starts: schedule emits [0,3,3,2] per ct so they
                        # tolerate the DVE chain's lag.
                        EMIT_AT = ((0, 0), (0, 3), (3, 6), (6, 8))
                        for ct in range(NCT):
                            if d == 0 and ti == 0 and ct == 0:
                                # startup is supply-bound: ck-major across
                                # k/v/r so each arriving ck quad feeds 3
                                # matmuls and ct0 finishes (and the DVE
                                # chain starts) as early as possible.
                                kvr = [pp.tile([128, TCH], F32, tag="proj",
                                               name="ps") for _ in range(3)]
                                for ck in range(8):
                                    for w_t, dst in zip((wk_t, wv_t, wr_t), kvr):
                                        nc.tensor.matmul(
                                            dst[:], w_t[:, ck, 0:128],
                                            x_t[:, ck],
                                            start=(ck == 0), stop=(ck == 7),
                                        )
                                k_ps, v_ps, r_ps = kvr
                            else:
                                k_ps, v_ps, r_ps = proj_matmuls(
                                    ct, x_t, rev, (wk_t, wv_t, wr_t))
                            z = elementwise(d, ti, ct, k_ps, v_ps, r_ps)
                            z_tiles.append(z)
                            if prev_out is not None:
                                emit_outproj(prev_out, *EMIT_AT[ct])
                        prev_out = (wo_t, z_tiles, outT_d, t0)
                    else:
                        # Final chunk: normal spread-emit of the previous
                        # chunk (keeps ACT copies and the po ring flowing),
                        # then accumulate this chunk's out-proj per-ct into
                        # 8 PSUM banks so the post-z3 tail is minimal.
                        EMIT_AT = ((0, 0), (0, 3), (3, 6), (6, 8))
                        z_tiles = []
                        for ct in range(NCT):
                            k_ps, v_ps, r_ps = proj_matmuls(
                                ct, x_t, rev, (wk_t, wv_t, wr_t))
                            z = elementwise(d, ti, ct, k_ps, v_ps, r_ps)
                            z_tiles.append(z)
                            emit_outproj(prev_out, *EMIT_AT[ct])
                        fin = []
                        for et in range(8):
                            pool = pp if et < 6 else po
                            tg = "proj" if et < 6 else "ops"
                            fin.append(pool.tile([128, TCH], F32, tag=tg,
                                                 name=f"fin{et}"))
                        for ct in range(NCT):
                            esl_w = wo_t[:, ct, :]
                            for et in range(8):
                                nc.tensor.matmul(
                                    fin[et][:],
                                    esl_w[:, et * 128:(et + 1) * 128],
                                    z_tiles[ct][:],
                                    start=(ct == 0), stop=(ct == NCT - 1),
                                )
                        for et in range(8):
                            o_sb = osb_pool.tile([128, TCH], OUT_DT, tag="osb")
                            if et % 2 == 0:
                                nc.scalar.copy(o_sb[:], fin[et][:])
                            else:
                                nc.vector.tensor_copy(o_sb[:], fin[et][:])
                            eng = nc.sync if et % 2 == 0 else nc.scalar
                            eng.dma_start(
                                outT_d[et * 128:(et + 1) * 128, t0:t0 + TCH],
                                o_sb[:])

    nc.compile()
    return nc


def _prep_inputs(x, rkv_w, out_w, time_decay, time_first, time_decay_rev, time_first_rev):
    """Host-side sharding + layout prep. Returns list of 8 input dicts."""
    import ml_dtypes
    f32 = np.float32
    bf16 = ml_dtypes.bfloat16
    mm_np = bf16 if MM_DT == BF16 else f32
    in_maps = []
    wd_f = -np.exp(time_decay.astype(np.float64))
    wd_b = -np.exp(time_decay_rev.astype(np.float64))
    lam_full_f = np.exp(wd_f).astype(f32)        # [C]
    lam_full_b = np.exp(wd_b).astype(f32)
    eu_full_f = np.exp(time_first.astype(np.float64)).astype(f32)
    eu_full_b = np.exp(time_first_rev.astype(np.float64)).astype(f32)

    for core in range(8):
        b, h = core // 2, core % 2
        cs = slice(h * H, h * H + H)
        xb = x[b].T.astype(f32)                                    # [C, T]
        # [NTC, 128, 8(ck), TCH]: c = ck*128 + p
        xtile = np.ascontiguousarray(
            xb.reshape(8, 128, NTC, TCH).transpose(2, 1, 0, 3)).astype(mm_np)
        def wtile(w):   # W^T [C, H] -> [128, 8(ck), H]
            return np.ascontiguousarray(
                w.reshape(8, 128, -1).transpose(1, 0, 2)).astype(mm_np)
        def wotile(w):  # [H, C] -> [128, NCT, C] bf16
            return np.ascontiguousarray(
                w.reshape(NCT, 128, -1).transpose(1, 0, 2)).astype(bf16)
        im = {
            "xT": xtile,
            "Wrf": wtile(rkv_w[0 * C:1 * C][cs].T.astype(f32)),
            "Wkf": wtile(rkv_w[1 * C:2 * C][cs].T.astype(f32)),
            "Wvf": wtile(rkv_w[2 * C:3 * C][cs].T.astype(f32)),
            "Wrb": wtile(rkv_w[3 * C:4 * C][cs].T.astype(f32)),
            "Wkb": wtile(rkv_w[4 * C:5 * C][cs].T.astype(f32)),
            "Wvb": wtile(rkv_w[5 * C:6 * C][cs].T.astype(f32)),
            "Wof": wotile((0.5 * out_w[:, cs].T).astype(f32)),
            "Wob": wotile((0.5 * out_w[:, C:][:, cs].T).astype(f32)),
        }
        for nm, lam_full, eu_full in (("f", lam_full_f, eu_full_f),
                                      ("b", lam_full_b, eu_full_b)):
            lam_loc = lam_full[cs]    # [H]
            eu_loc = eu_full[cs]
            lam_tile = np.empty((128, NCT * TCH), f32)
            eu_tile = np.empty((128, NCT), f32)
            for ct in range(NCT):
                lam_tile[:, ct * TCH:(ct + 1) * TCH] = lam_loc[ct * 128:(ct + 1) * 128][:, None]
                eu_tile[:, ct] = eu_loc[ct * 128:(ct + 1) * 128]
            im["lam" + nm] = lam_tile
            im["eu" + nm] = eu_tile
        in_maps.append(im)
    return in_maps


def run(inputs, trace=False, tmpdir=None):
    global _compiled
    if _compiled is None:
        _compiled = _build()
    in_maps = _prep_inputs(**inputs)
    tcores = None
    if os.environ.get("BIRWKV_TRACE_ALL"):
        tcores = list(range(8))
    res = run_bass_kernel_spmd(_compiled, in_maps, list(range(8)),
                               trace=trace, tmpdir=tmpdir, trace_cores=tcores)
    out = np.zeros((B, T, C), np.float32)
    for core in range(8):
        b = core // 2
        r = res.results[core]
        out[b] += np.asarray(r["outTf"]).astype(np.float32).T
        out[b] += np.asarray(r["outTb"]).astype(np.float32).T[::-1]
    return out, res


def kernel(**inputs):
    out, _ = run(inputs)
    return out


# revision 46
# speedup vs baseline: 1.0497x; 1.0497x over previous
"""BiRWKV layer kernel for Trainium2 (8 NeuronCores, Bass/Tile).

Problem: x[4,2048,1024] -> rkv = x @ rkv_w.T -> (r,k,v) fwd + bwd,
WKV scan per direction, gate with sigmoid(r), concat, out @ out_w.T.

Strategy (v2):
  - Shard over (batch b, channel-half h): core = 2*b + h. Each core handles
    one batch's 512 fwd + 512 bwd channels end-to-end.
  - Channels-on-partitions layout [c, t]: projections via PE matmul
    (lhsT = W^T tile [c,d], rhs = x^T [c,t]), WKV recurrence via the DVE's
    native tensor_tensor_scan, out-projection partial via PE (contraction
    over local c), summed across cores on host.
  - Unstabilized scan: A_t = lam*A + e^k v, D_t = lam*D + e^k,
    y = (A_{t-1} + e^u e^k v)/(D_{t-1} + e^u e^k). fp32 never overflows for
    this problem's ranges; matches the stabilized reference to ~1e-7.
  - Backward direction = forward scan on time-reversed rhs (stride trick).
  - sigmoid(r)*y = 0.5*(1+tanh(r/2))*y; the 0.5 is folded into out_w.

  v2 performance changes vs v1:
  - x loaded ONCE and kept resident in SBUF across both directions
    (3-buffer ring; only time-chunk 0 is reloaded for the backward pass).
  - Input DMAs batched (1 dispatch/tensor; per-ck only for startup-critical
    tiles) and issued from the ACT engine (own HWDGE queue); output DMAs
    stay on the sync engine queue -> 2 parallel DMA queues.
  - exp/tanh/PSUM-copies on ACT; the fp32 elementwise chain stays on DVE
    (GpSimd only gets tiny carry copies/memsets: its ISA lacks STT/divide,
    its TT ops run at ~2x the DVE cost, and keeping it busy triggers
    power throttling that slows every other engine ~25%).
  - Out-projection in bf16 (Wo, z, outputs) - halves those transfers;
    output partials written bf16, upconverted on host.
  - Final-chunk out-projection accumulates over ct into 8 PSUM banks so the
    tail after the last z is ~8 matmuls instead of a full 32-matmul flush.
"""
import os
import sys
import numpy as np

sys.path.insert(0, "/opt/trn_rl_repo")

import concourse.bass as bass
import concourse.mybir as mybir
from concourse import bacc
import concourse.tile as tile
from concourse.bass_utils import run_bass_kernel_spmd

B, T, C = 4, 2048, 1024
H = C // 2          # channels per core per direction (512)
NCT = H // 128      # c-tiles per direction (4)
TCH = 512           # time chunk
NTC = T // TCH      # t-chunks (4)
F32 = mybir.dt.float32
BF16 = mybir.dt.bfloat16
F32R = mybir.dt.float32r
F8 = mybir.dt.float8e4
AF = mybir.ActivationFunctionType
ALU = mybir.AluOpType
DR = mybir.MatmulPerfMode.DoubleRow
# r-weights are scaled x16 on the host (keeps fp8e4m3 out of subnormals);
# folded back via the tanh activation scale (0.5 / 16).
R_SCALE = 16.0

MM_DT = {"f32": F32, "f32r": F32R, "bf16": BF16}[
    os.environ.get("BIRWKV_MM_DT", "bf16")]
WO_DT = BF16
OUT_DT = BF16

_compiled = None


def _build():
    nc = bacc.Bacc("TRN2", target_bir_lowering=False, debug=False, num_devices=8)

    xT = nc.dram_tensor("xT", [NTC, 128, 8, TCH], MM_DT, kind="ExternalInput").ap()
    x8T = nc.dram_tensor("x8T", [NTC, 128, 8, TCH], F8, kind="ExternalInput").ap()
    Wkf = nc.dram_tensor("Wkf", [128, 8, H], MM_DT, kind="ExternalInput").ap()
    Wvf = nc.dram_tensor("Wvf", [128, 8, H], MM_DT, kind="ExternalInput").ap()
    Wrf = nc.dram_tensor("Wrf", [128, 8, H], F8, kind="ExternalInput").ap()
    Wkb = nc.dram_tensor("Wkb", [128, 8, H], MM_DT, kind="ExternalInput").ap()
    Wvb = nc.dram_tensor("Wvb", [128, 8, H], MM_DT, kind="ExternalInput").ap()
    Wrb = nc.dram_tensor("Wrb", [128, 8, H], F8, kind="ExternalInput").ap()
    Wof = nc.dram_tensor("Wof", [128, NCT, C], WO_DT, kind="ExternalInput").ap()
    Wob = nc.dram_tensor("Wob", [128, NCT, C], WO_DT, kind="ExternalInput").ap()
    lamf = nc.dram_tensor("lamf", [128, NCT * TCH], F32, kind="ExternalInput").ap()
    lamb = nc.dram_tensor("lamb", [128, NCT * TCH], F32, kind="ExternalInput").ap()
    euf = nc.dram_tensor("euf", [128, NCT], F32, kind="ExternalInput").ap()
    eub = nc.dram_tensor("eub", [128, NCT], F32, kind="ExternalInput").ap()

    outTf = nc.dram_tensor("outTf", [C, T], OUT_DT, kind="ExternalOutput").ap()
    outTb = nc.dram_tensor("outTb", [C, T], OUT_DT, kind="ExternalOutput").ap()

    with tile.TileContext(nc) as tc:
        with (
            tc.tile_pool(name="xp", bufs=1) as xp_pool,
            tc.tile_pool(name="wk", bufs=2) as wk_pool,
            tc.tile_pool(name="wv", bufs=2) as wv_pool,
            tc.tile_pool(name="wr", bufs=2) as wr_pool,
            tc.tile_pool(name="wo", bufs=2) as wo_pool,
            tc.tile_pool(name="lam", bufs=1) as lam_pool,
            tc.tile_pool(name="ew", bufs=2) as ew_pool,
            tc.tile_pool(name="ab", bufs=1) as ab_pool,
            tc.tile_pool(name="zs", bufs=2) as z_pool,
            tc.tile_pool(name="osb", bufs=6) as osb_pool,
            tc.tile_pool(name="pp", bufs=6, space="PSUM") as pp,
            tc.tile_pool(name="po", bufs=2, space="PSUM") as po,
        ):
            # ---- startup DMAs ------------------------------------------
            # x chunks live in a 3-buffer ring (tags xA/xB/xC; time-chunks
            # 0 and 3 share xA).  Chunk 0 + Wkf are the first-matmul
            # critical path: per-ck dispatches (subtile deps let ck0's
            # matmul start while ck1.. are in flight), interleaved on the
            # sync queue.  Wvf/Wrf per-ck on the ACT queue in parallel.
            x_cur = {}
            x_cur[0] = xp_pool.tile([128, 8, TCH], MM_DT, tag="xA", name="x0")
            x_cur[1] = xp_pool.tile([128, 8, TCH], MM_DT, tag="xB", name="x1")
            x_cur[2] = xp_pool.tile([128, 8, TCH], MM_DT, tag="xC", name="x2")
            x8_cur = {}
            for ti in range(NTC):
                x8_cur[ti] = xp_pool.tile([128, 8, TCH], F8, tag=f"x8{ti}",
                                          name=f"x8{ti}")
            wk_t = wk_pool.tile([128, 8, H], MM_DT, tag="wk")
            wv_t = wv_pool.tile([128, 8, H], MM_DT, tag="wv")
            wr_t = wr_pool.tile([128, 8, H], F8, tag="wr")
            wo_t = wo_pool.tile([128, NCT, C], WO_DT, tag="wo")
            for ck in range(8):
                nc.sync.dma_start(wk_t[:, ck], Wkf[:, ck])
                nc.sync.dma_start(x_cur[0][:, ck], xT[0, :, ck])
            for ck in range(8):
                nc.scalar.dma_start(wv_t[:, ck], Wvf[:, ck])
            nc.scalar.dma_start(wr_t[:], Wrf[:])
            nc.scalar.dma_start(x8_cur[0][:], x8T[0])
            # queue order = arrival order.  qAct: wv, wr (ck-streamed,
            # above), wo (~32us), x2 (~55us), lam1/eu1 (backward pass).
            # qSP: wk/x0 (critical), lam0/eu0 (~13us), x1 (~28us), x3.
            nc.scalar.dma_start(wo_t[:], Wof[:])
            lam_t = {}
            eu_t = {}
            lam_t[0] = lam_pool.tile([128, NCT * TCH], F32, tag="lam0", name="lam0")
            nc.sync.dma_start(lam_t[0][:], lamf[:])
            eu_t[0] = lam_pool.tile([128, NCT], F32, tag="eu0", name="eu0")
            nc.sync.dma_start(eu_t[0][:], euf[:])
            nc.sync.dma_start(x_cur[1][:], xT[1])
            nc.sync.dma_start(x8_cur[1][:], x8T[1])
            nc.scalar.dma_start(x_cur[2][:], xT[2])
            nc.scalar.dma_start(x8_cur[2][:], x8T[2])
            nc.sync.dma_start(x8_cur[3][:], x8T[3])
            lam_t[1] = lam_pool.tile([128, NCT * TCH], F32, tag="lam1", name="lam1")
            nc.scalar.dma_start(lam_t[1][:], lamb[:])
            eu_t[1] = lam_pool.tile([128, NCT], F32, tag="eu1", name="eu1")
            nc.scalar.dma_start(eu_t[1][:], eub[:])
            x_cur[3] = xp_pool.tile([128, 8, TCH], MM_DT, tag="xD", name="x3")
            nc.sync.dma_start(x_cur[3][:], xT[3])

            def emit_outproj(prev, e0, e1):
                wo_p, z_tiles, outT_p, t0 = prev
                for et in range(e0, e1):
                    esl = slice(et * 128, (et + 1) * 128)
                    o_ps = po.tile([128, TCH], F32, tag="ops")
                    for ct in range(NCT):
                        nc.tensor.matmul(
                            o_ps[:],
                            wo_p[:, ct, esl],
                            z_tiles[ct][:],
                            start=(ct == 0), stop=(ct == NCT - 1),
                        )
                    o_sb = osb_pool.tile([128, TCH], OUT_DT, tag="osb")
                    nc.scalar.copy(o_sb[:], o_ps[:])
                    nc.sync.dma_start(outT_p[et * 128:(et + 1) * 128, t0:t0 + TCH], o_sb[:])

            def elementwise(d, ti, ct, k_ps, v_ps, r_ps):
                """exp/tanh on ACT; pv+scans+num/den+recip on DVE (GpSimd
                has no scalar_tensor_tensor); y + gating + carries on
                GpSimd. Returns the z tile (bf16)."""
                p = ew_pool.tile([128, TCH], F32, tag="p")
                nc.scalar.activation(p[:], k_ps[:], AF.Exp)
                th = ew_pool.tile([128, TCH], F32, tag="th")
                nc.scalar.activation(th[:], r_ps[:], AF.Tanh, scale=0.5 / R_SCALE)
                pv = ew_pool.tile([128, TCH], F32, tag="pv")
                nc.vector.tensor_mul(pv[:], p[:], v_ps[:])

                a_buf = ab_pool.tile([128, TCH + 1], F32, tag=f"A{ct}", name=f"A{ct}")
                d_buf = ab_pool.tile([128, TCH + 1], F32, tag=f"D{ct}", name=f"D{ct}")
                if ti == 0:
                    nc.gpsimd.memset(a_buf[:, 0:1], 0.0)
                    nc.gpsimd.memset(d_buf[:, 0:1], 0.0)
                else:
                    nc.gpsimd.tensor_copy(a_buf[:, 0:1], a_buf[:, TCH:TCH + 1])
                    nc.gpsimd.tensor_copy(d_buf[:, 0:1], d_buf[:, TCH:TCH + 1])
                lam_sl = lam_t[d][:, ct * TCH:(ct + 1) * TCH]
                nc.vector.tensor_tensor_scan(
                    a_buf[:, 1:TCH + 1], lam_sl, pv[:],
                    a_buf[:, 0:1], ALU.mult, ALU.add)
                nc.vector.tensor_tensor_scan(
                    d_buf[:, 1:TCH + 1], lam_sl, p[:],
                    d_buf[:, 0:1], ALU.mult, ALU.add)

                eu_sl = eu_t[d][:, ct:ct + 1]
                num = ew_pool.tile([128, TCH], F32, tag="num")
                nc.vector.scalar_tensor_tensor(
                    num[:], pv[:], eu_sl, a_buf[:, 0:TCH], ALU.mult, ALU.add)
                den = ew_pool.tile([128, TCH], F32, tag="den")
                nc.vector.scalar_tensor_tensor(
                    den[:], p[:], eu_sl, d_buf[:, 0:TCH], ALU.mult, ALU.add)
                rec = ew_pool.tile([128, TCH], F32, tag="rec")
                nc.vector.reciprocal_approx_fast(rec[:], den[:])
                y = ew_pool.tile([128, TCH], F32, tag="y")
                nc.vector.tensor_mul(y[:], num[:], rec[:])
                z = z_pool.tile([128, TCH], WO_DT, tag=f"z{ct}", name=f"z{ct}")
                nc.vector.scalar_tensor_tensor(
                    z[:], th[:], 1.0, y[:], ALU.add, ALU.mult)
                return z

            def proj_matmuls(ct, x_t, rev, wts):
                dsl = slice(ct * 128, (ct + 1) * 128)
                ps = []
                for w_t in wts:
                    dst = pp.tile([128, TCH], F32, tag="proj", name="ps")
                    for ck in range(8):
                        rhs = x_t[:, ck]
                        if rev:
                            rhs = rhs[:, ::-1]
                        nc.tensor.matmul(
                            dst[:], w_t[:, ck, dsl], rhs,
                            start=(ck == 0), stop=(ck == 7),
                        )
                    ps.append(dst)
                return ps

            def r_proj_dr(ct, x8_t, rev, wr8_t):
                """r-projection as 4 fp8 DoubleRow matmuls over ck-pairs:
                lhsT [128,(j:2),(m:128)], rhs [128,(j:2),(t:TCH)],
                out [128(m), TCH] = sum over p and j."""
                dsl = slice(ct * 128, (ct + 1) * 128)
                dst = pp.tile([128, TCH], F32, tag="proj", name="ps")
                for q in range(4):
                    jsl = slice(2 * q, 2 * q + 2)
                    rhs = x8_t[:, jsl, :]
                    if rev:
                        rhs = rhs[:, :, ::-1]
                    nc.tensor.matmul(
                        dst[:], wr8_t[:, jsl, dsl], rhs,
                        start=(q == 0), stop=(q == 3), perf_mode=DR,
                    )
                return dst

            prev_out = None
            for d, (Wk, Wv, Wr, Wo, outT_d) in enumerate((
                (Wkf, Wvf, Wrf, Wof, outTf),
                (Wkb, Wvb, Wrb, Wob, outTb),
            )):
                rev = (d == 1)
                if d == 1:
                    # all weights were prefetched into their second ring
                    # slots mid-forward-pass.
                    wk_t = next_w["wk"]
                    wv_t = next_w["wv"]
                    wr_t = next_w["wr"]
                    wo_t = next_w["wo"]

                for ti in range(NTC):
                    t0 = ti * TCH
                    tis = NTC - 1 - ti if rev else ti
                    if d == 0 and ti == 2:
                        # prefetch all backward-pass weights into their free
                        # second ring slots (no dependency wait) so the
                        # pass transition starts with weights in SBUF.
                        next_w = {
                            "wk": wk_pool.tile([128, 8, H], MM_DT, tag="wk",
                                               name="wk2"),
                            "wv": wv_pool.tile([128, 8, H], MM_DT, tag="wv",
                                               name="wv2"),
                            "wr": wr_pool.tile([128, 8, H], F8, tag="wr",
                                               name="wr2"),
                            "wo": wo_pool.tile([128, NCT, C], WO_DT, tag="wo",
                                               name="wo2"),
                        }
                        nc.scalar.dma_start(next_w["wk"][:], Wkb[:])
                        nc.scalar.dma_start(next_w["wv"][:], Wvb[:])
                        nc.scalar.dma_start(next_w["wr"][:], Wrb[:])
                        nc.scalar.dma_start(next_w["wo"][:], Wob[:])
                    x_t = x_cur[tis]
                    x8_t = x8_cur[tis]
                    final = (d == 1 and ti == NTC - 1)

                    if not final:
                        z_tiles = []
                        # Prev-chunk emits need ALL of the previous z
                        # tiles, and prev-z3 lands barely before this chunk
                        # starts: schedule emits [0,3,3,2] per ct so they
                        # tolerate the DVE chain's lag.
                        EMIT_AT = ((0, 0), (0, 3), (3, 6), (6, 8))
                        for ct in range(NCT):
                            k_ps, v_ps = proj_matmuls(
                                ct, x_t, rev, (wk_t, wv_t))
                            r_ps = r_proj_dr(ct, x8_t, rev, wr_t)
                            z = elementwise(d, ti, ct, k_ps, v_ps, r_ps)
                            z_tiles.append(z)
                            if prev_out is not None:
                                emit_outproj(prev_out, *EMIT_AT[ct])
                        prev_out = (wo_t, z_tiles, outT_d, t0)
                    else:
                        # Final chunk: normal spread-emit of the previous
                        # chunk (keeps ACT copies and the po ring flowing),
                        # then accumulate this chunk's out-proj per-ct into
                        # 8 PSUM banks so the post-z3 tail is minimal.
                        EMIT_AT = ((0, 0), (0, 3), (3, 6), (6, 8))
                        z_tiles = []
                        for ct in range(NCT):
                            k_ps, v_ps = proj_matmuls(
                                ct, x_t, rev, (wk_t, wv_t))
                            r_ps = r_proj_dr(ct, x8_t, rev, wr_t)
                            z = elementwise(d, ti, ct, k_ps, v_ps, r_ps)
                            z_tiles.append(z)
                            emit_outproj(prev_out, *EMIT_AT[ct])
                        fin = []
                        for et in range(8):
                            pool = pp if et < 6 else po
                            tg = "proj" if et < 6 else "ops"
                            fin.append(pool.tile([128, TCH], F32, tag=tg,
                                                 name=f"fin{et}"))
                        for ct in range(NCT):
                            esl_w = wo_t[:, ct, :]
                            for et in range(8):
                                nc.tensor.matmul(
                                    fin[et][:],
                                    esl_w[:, et * 128:(et + 1) * 128],
                                    z_tiles[ct][:],
                                    start=(ct == 0), stop=(ct == NCT - 1),
                                )
                        for et in range(8):
                            o_sb = osb_pool.tile([128, TCH], OUT_DT, tag="osb")
                            if et % 2 == 0:
                                nc.scalar.copy(o_sb[:], fin[et][:])
                            else:
                                nc.vector.tensor_copy(o_sb[:], fin[et][:])
                            eng = nc.sync if et % 2 == 0 else nc.scalar
                            eng.dma_start(
                                outT_d[et * 128:(et + 1) * 128, t0:t0 + TCH],
                                o_sb[:])

    nc.compile()
    return nc


def _prep_inputs(x, rkv_w, out_w, time_decay, time_first, time_decay_rev, time_first_rev):
    """Host-side sharding + layout prep. Returns list of 8 input dicts."""
    import ml_dtypes
    f32 = np.float32
    bf16 = ml_dtypes.bfloat16
    mm_np = bf16 if MM_DT == BF16 else f32
    in_maps = []
    wd_f = -np.exp(time_decay.astype(np.float64))
    wd_b = -np.exp(time_decay_rev.astype(np.float64))
    lam_full_f = np.exp(wd_f).astype(f32)        # [C]
    lam_full_b = np.exp(wd_b).astype(f32)
    eu_full_f = np.exp(time_first.astype(np.float64)).astype(f32)
    eu_full_b = np.exp(time_first_rev.astype(np.float64)).astype(f32)

    for core in range(8):
        b, h = core // 2, core % 2
        cs = slice(h * H, h * H + H)
        xb = x[b].T.astype(f32)                                    # [C, T]
        # [NTC, 128, 8(ck), TCH]: c = ck*128 + p
        xtile_f = np.ascontiguousarray(
            xb.reshape(8, 128, NTC, TCH).transpose(2, 1, 0, 3))
        xtile = xtile_f.astype(mm_np)
        x8tile = xtile_f.astype(ml_dtypes.float8_e4m3)
        def wtile(w, dt=None):   # W^T [C, H] -> [128, 8(ck), H]
            return np.ascontiguousarray(
                w.reshape(8, 128, -1).transpose(1, 0, 2)).astype(dt or mm_np)
        def wotile(w):  # [H, C] -> [128, NCT, C] bf16
            return np.ascontiguousarray(
                w.reshape(NCT, 128, -1).transpose(1, 0, 2)).astype(bf16)
        f8 = ml_dtypes.float8_e4m3
        im = {
            "xT": xtile,
            "x8T": x8tile,
            "Wrf": wtile(16.0 * rkv_w[0 * C:1 * C][cs].T.astype(f32), f8),
            "Wkf": wtile(rkv_w[1 * C:2 * C][cs].T.astype(f32)),
            "Wvf": wtile(rkv_w[2 * C:3 * C][cs].T.astype(f32)),
            "Wrb": wtile(16.0 * rkv_w[3 * C:4 * C][cs].T.astype(f32), f8),
            "Wkb": wtile(rkv_w[4 * C:5 * C][cs].T.astype(f32)),
            "Wvb": wtile(rkv_w[5 * C:6 * C][cs].T.astype(f32)),
            "Wof": wotile((0.5 * out_w[:, cs].T).astype(f32)),
            "Wob": wotile((0.5 * out_w[:, C:][:, cs].T).astype(f32)),
        }
        for nm, lam_full, eu_full in (("f", lam_full_f, eu_full_f),
                                      ("b", lam_full_b, eu_full_b)):
            lam_loc = lam_full[cs]    # [H]
            eu_loc = eu_full[cs]
            lam_tile = np.empty((128, NCT * TCH), f32)
            eu_tile = np.empty((128, NCT), f32)
            for ct in range(NCT):
                lam_tile[:, ct * TCH:(ct + 1) * TCH] = lam_loc[ct * 128:(ct + 1) * 128][:, None]
                eu_tile[:, ct] = eu_loc[ct * 128:(ct + 1) * 128]
            im["lam" + nm] = lam_tile
            im["eu" + nm] = eu_tile
        in_maps.append(im)
    return in_maps


def run(inputs, trace=False, tmpdir=None):
    global _compiled
    if _compiled is None:
        _compiled = _build()
    in_maps = _prep_inputs(**inputs)
    tcores = None
    if os.environ.get("BIRWKV_TRACE_ALL"):
        tcores = list(range(8))
    res = run_bass_kernel_spmd(_compiled, in_maps, list(range(8)),
                               trace=trace, tmpdir=tmpdir, trace_cores=tcores)
    out = np.zeros((B, T, C), np.float32)
    for core in range(8):
        b = core // 2
        r = res.results[core]
        out[b] += np.asarray(r["outTf"]).astype(np.float32).T
        out[b] += np.asarray(r["outTb"]).astype(np.float32).T[::-1]
    return out, res


def kernel(**inputs):
    out, _ = run(inputs)
    return out
